# revision 6
# baseline (speedup 1.0000x reference)
"""NGCF layer (gather + segment_sum + dense epilogue) on 8 Trainium2 cores.

Strategy:
- Shard destination nodes (rows) across 8 cores: core c owns rows
  [c*12500, (c+1)*12500).
- Per core, edges are grouped by (source col chunk of 25088 rows,
  dest window of 128 rows), padded per cell to a multiple of 128 and to
  a cross-core-uniform tile count (SPMD: one program, per-core data).
- Messages are gathered from HBM via the custom dma_gather instruction
  (int16 indices per 25088-row chunk table, 1024-index ops spread over
  4 SWDGE queues for parallel descriptor generation).
- Each 128-edge tile is scattered into its window via a PE matmul with a
  DVE-built one-hot matrix S[e, r] = val_e * (rl_e == r); windows
  accumulate in PSUM and flush into an SBUF accumulator holding
  neighbor^T [128 dims, 12500 rows].
- Dense epilogue per window, all in transposed layout:
  out^T = leaky_relu(W1^T.T @ X^T + W2^T.T @ N^T + W2^T.T @ ((W2^T.T @ N^T) * X^T) + bias)
- Host assembles the full [100000, 128] output.
"""

import sys, os

sys.path.insert(0, "/opt/trn_rl_repo")

import numpy as np

N_NODES = 100000
N_EDGES = 1600000
D = 128
N_CORES = 8
ROWS_PER_CORE = N_NODES // N_CORES          # 12500
WIN = 128                                    # dest rows per window
N_WIN = (ROWS_PER_CORE + WIN - 1) // WIN     # 98 (last window 84 rows)
CHUNK = 25088                                # source rows per gather table chunk
N_CHUNK = (N_NODES + CHUNK - 1) // CHUNK     # 4
TILE = 128                                   # edges per matmul tile
OP_TILES = 8                                 # tiles per dma_gather op (1024 idx)


def _prep(embeddings, adj_vals, adj_rows, adj_cols):
    """Host-side: shard + sort edges, build padded per-core streams."""
    rows = np.asarray(adj_rows).astype(np.int64)
    cols = np.asarray(adj_cols).astype(np.int64)
    vals = np.asarray(adj_vals).astype(np.float32)

    core = rows // ROWS_PER_CORE
    rows_l = rows - core * ROWS_PER_CORE
    win = rows_l // WIN
    chunk = cols // CHUNK
    cell = chunk * N_WIN + win                       # cell id within a core
    n_cells = N_CHUNK * N_WIN

    per_core = []
    cnt = np.zeros((N_CORES, n_cells), dtype=np.int64)
    for c in range(N_CORES):
        m = core == c
        order = np.lexsort((rows_l[m], cell[m]))
        e = {
            "cell": cell[m][order],
            "rl": (rows_l[m] - win[m] * WIN)[order].astype(np.float32),
            "idx": (cols[m] - chunk[m] * CHUNK)[order].astype(np.int16),
            "val": vals[m][order],
        }
        cnt[c] = np.bincount(e["cell"], minlength=n_cells)
        per_core.append(e)

    # uniform cell capacities (tiles) across cores
    tiles_per_cell = np.maximum(1, -(-cnt.max(axis=0) // TILE))   # [n_cells]
    cap = tiles_per_cell * TILE
    cell_start = np.concatenate([[0], np.cumsum(cap)])            # [n_cells+1]
    total = int(cell_start[-1])                                   # padded edges/core
    tt = total // TILE                                            # total tiles/core

    # per-chunk tile counts and gather op sizes (same for all cores)
    chunk_tiles = [int(tiles_per_cell[ch * N_WIN:(ch + 1) * N_WIN].sum())
                   for ch in range(N_CHUNK)]
    ops = []   # (chunk, tile_offset_global, n_tiles)
    toff = 0
    for ch in range(N_CHUNK):
        left = chunk_tiles[ch]
        while left > 0:
            n = min(OP_TILES, left)
            ops.append((ch, toff, n))
            toff += n
            left -= n
    assert toff == tt

    # per-tile cell id / flags (same for all cores)
    tile_cell = np.repeat(np.arange(n_cells), tiles_per_cell)
    first = np.zeros(tt, dtype=bool)
    last = np.zeros(tt, dtype=bool)
    pos = 0
    for ci in range(n_cells):
        k = tiles_per_cell[ci]
        first[pos] = True
        last[pos + k - 1] = True
        pos += k

    # per-core padded streams
    idx_streams, rl_all, val_all = [], [], []
    for c in range(N_CORES):
        e = per_core[c]
        n = len(e["cell"])
        # slot of each real edge: cell_start[cell] + rank within cell
        cell_first = np.searchsorted(e["cell"], np.arange(n_cells), side="left")
        rank = np.arange(n) - cell_first[e["cell"]]
        slot = cell_start[e["cell"]] + rank
        idx_s = np.zeros(total, dtype=np.int16)
        rl_s = np.zeros(total, dtype=np.float32)
        val_s = np.zeros(total, dtype=np.float32)
        idx_s[slot] = e["idx"]
        rl_s[slot] = e["rl"]
        val_s[slot] = e["val"]
        idx_streams.append(idx_s)
        rl_all.append(rl_s.reshape(tt, TILE).T.copy())      # [128, tt]
        val_all.append(val_s.reshape(tt, TILE).T.copy())    # [128, tt]

    # idx arrays in the dma_gather wrapped layout, one column-block per op
    idx_wrapped = []
    for c in range(N_CORES):
        parts = []
        for (ch, toff2, ntile) in ops:
            chunk_idx = idx_streams[c][toff2 * TILE:(toff2 + ntile) * TILE]
            parts.append(chunk_idx.reshape(-1, 16).T)       # [16, ntile*8]
        w16 = np.concatenate(parts, axis=1)                 # [16, tt*8]
        idx_wrapped.append(np.tile(w16, (8, 1)))            # [128, tt*8]

    meta = {
        "tt": tt,
        "ops": ops,
        "tile_cell": tile_cell,
        "first": first,
        "last": last,
        "tiles_per_cell": tiles_per_cell,
    }
    return meta, idx_wrapped, rl_all, val_all


_BUILD_CACHE = {}


def _build(meta, chunk_rows):
    import concourse.bacc as bacc
    import concourse.bass as bass
    import concourse.mybir as mybir
    import concourse.tile as tile
    from concourse.library_config import mlp

    tt = meta["tt"]
    ops = meta["ops"]
    tile_cell = meta["tile_cell"]
    first = meta["first"]
    last = meta["last"]

    nc = bacc.Bacc(None, target_bir_lowering=False, num_swdge_queues=4)
    f32 = mybir.dt.float32
    embc = [nc.dram_tensor(f"embc{ch}", [chunk_rows[ch], D], f32, kind="ExternalInput")
            for ch in range(N_CHUNK)]
    idx_t = nc.dram_tensor("idx", [128, tt * 8], mybir.dt.int16, kind="ExternalInput")
    rl_t = nc.dram_tensor("rl", [128, tt], f32, kind="ExternalInput")
    val_t = nc.dram_tensor("val", [128, tt], f32, kind="ExternalInput")
    embt_t = nc.dram_tensor("embt", [128, ROWS_PER_CORE], f32, kind="ExternalInput")
    w1t_t = nc.dram_tensor("w1t", [D, D], f32, kind="ExternalInput")
    w2t_t = nc.dram_tensor("w2t", [D, D], f32, kind="ExternalInput")
    bias_t = nc.dram_tensor("bias", [D, 1], f32, kind="ExternalInput")
    b2_t = nc.dram_tensor("b2v", [D, 1], f32, kind="ExternalInput")
    outt_t = nc.dram_tensor("outt", [128, ROWS_PER_CORE], f32, kind="ExternalOutput")

    eq = mybir.AluOpType.is_equal
    mult = mybir.AluOpType.mult
    addop = mybir.AluOpType.add
    maxop = mybir.AluOpType.max

    with tile.TileContext(nc) as tc:
        with (
            tc.tile_pool(name="const", bufs=1) as constp,
            tc.tile_pool(name="gath", bufs=4) as gathp,
            tc.tile_pool(name="stp", bufs=3) as stp,
            tc.tile_pool(name="pcell", bufs=4, space="PSUM") as pcell,
            tc.tile_pool(name="pepi", bufs=2, space="PSUM") as pepi,
            tc.tile_pool(name="episb", bufs=3) as episb,
        ):
            nc.gpsimd.load_library(mlp)

            idx_sb = constp.tile([128, tt * 8], mybir.dt.int16)
            nc.sync.dma_start(out=idx_sb[:], in_=idx_t[:])
            rl_sb = constp.tile([128, tt], f32)
            nc.sync.dma_start(out=rl_sb[:], in_=rl_t[:])
            val_sb = constp.tile([128, tt], f32)
            nc.sync.dma_start(out=val_sb[:], in_=val_t[:])
            w1t_sb = constp.tile([D, D], f32)
            nc.sync.dma_start(out=w1t_sb[:], in_=w1t_t[:])
            w2t_sb = constp.tile([D, D], f32)
            nc.sync.dma_start(out=w2t_sb[:], in_=w2t_t[:])
            bias_sb = constp.tile([D, 1], f32)
            nc.sync.dma_start(out=bias_sb[:], in_=bias_t[:])
            b2_sb = constp.tile([D, 1], f32)
            nc.sync.dma_start(out=b2_sb[:], in_=b2_t[:])

            acc = constp.tile([128, ROWS_PER_CORE], f32)

            # iota 0..127 repeated OP_TILES times along free: [128, 8, 128]
            iota_i = constp.tile([128, OP_TILES * TILE], mybir.dt.int32)
            nc.gpsimd.iota(iota_i[:], pattern=[[0, OP_TILES], [1, TILE]],
                           base=0, channel_multiplier=0)
            iota_f = constp.tile([128, OP_TILES * TILE], f32)
            nc.vector.tensor_copy(out=iota_f[:], in_=iota_i[:])

            def epilogue(w):
                lo = w * WIN
                wlen = min(WIN, ROWS_PER_CORE - lo)
                xs_t = episb.tile([128, WIN], f32, tag="xs")
                nc.sync.dma_start(out=xs_t[:, :wlen], in_=embt_t[:, lo:lo + wlen])
                xs = xs_t[:, :wlen]
                pa = pepi.tile([128, WIN], f32, tag="pa")
                nc.tensor.matmul(out=pa[:, :wlen], lhsT=w2t_sb[:],
                                 rhs=acc[:, lo:lo + wlen], start=True, stop=True)
                it = episb.tile([128, WIN], f32, tag="it")
                nc.vector.scalar_tensor_tensor(out=it[:, :wlen], in0=pa[:, :wlen],
                                               scalar=b2_sb[:], in1=xs,
                                               op0=addop, op1=mult)
                pb = pepi.tile([128, WIN], f32, tag="pb")
                nc.tensor.matmul(out=pb[:, :wlen], lhsT=w1t_sb[:], rhs=xs,
                                 start=True, stop=False)
                nc.tensor.matmul(out=pb[:, :wlen], lhsT=w2t_sb[:],
                                 rhs=acc[:, lo:lo + wlen], start=False, stop=False)
                nc.tensor.matmul(out=pb[:, :wlen], lhsT=w2t_sb[:],
                                 rhs=it[:, :wlen], start=False, stop=True)
                s2 = episb.tile([128, WIN], f32, tag="s2")
                nc.vector.tensor_scalar_add(s2[:, :wlen], pb[:, :wlen], bias_sb[:])
                t1 = episb.tile([128, WIN], f32, tag="t1")
                nc.scalar.mul(t1[:, :wlen], s2[:, :wlen], 0.2)
                o = episb.tile([128, WIN], f32, tag="o")
                nc.vector.tensor_tensor(out=o[:, :wlen], in0=s2[:, :wlen],
                                        in1=t1[:, :wlen], op=maxop)
                nc.sync.dma_start(out=outt_t[:, lo:lo + wlen], in_=o[:, :wlen])

            cur_psum = None
            qn = 0
            for (ch, toff, ntile) in ops:
                n_idx = ntile * TILE
                dst = gathp.tile([128, OP_TILES, D], f32, tag="dst")
                nc.gpsimd.dma_gather(
                    dst[:, :ntile, :],
                    embc[ch][:],
                    idx_sb[:, toff * 8:toff * 8 + n_idx // 16],
                    n_idx, n_idx, D,
                    single_packet=True,
                    queue_num=qn % 4,
                )
                qn += 1
                # one wide S build for the whole op
                st_big = stp.tile([128, OP_TILES * TILE], f32, tag="st")
                rl_b = rl_sb[:, toff:toff + ntile].to_broadcast(
                    (128, ntile, TILE))
                val_b = val_sb[:, toff:toff + ntile].to_broadcast(
                    (128, ntile, TILE))
                eqv = stp.tile([128, OP_TILES * TILE], f32, tag="eqv")
                nc.vector.tensor_tensor(
                    out=eqv[:, :n_idx].rearrange("p (a b) -> p a b", b=TILE),
                    in0=iota_f[:, :n_idx].rearrange("p (a b) -> p a b", b=TILE),
                    in1=rl_b, op=eq)
                nc.vector.tensor_tensor(
                    out=st_big[:, :n_idx].rearrange("p (a b) -> p a b", b=TILE),
                    in0=eqv[:, :n_idx].rearrange("p (a b) -> p a b", b=TILE),
                    in1=val_b, op=mult)

                for j in range(ntile):
                    t = toff + j
                    ci = int(tile_cell[t])
                    w = ci % N_WIN
                    if first[t]:
                        cur_psum = pcell.tile([128, WIN], f32, tag="pc")
                    nc.tensor.matmul(
                        out=cur_psum[:],
                        lhsT=dst[:, j, :],
                        rhs=st_big[:, j * TILE:(j + 1) * TILE],
                        start=bool(first[t]), stop=bool(last[t]),
                    )
                    if last[t]:
                        lo = w * WIN
                        wlen = min(WIN, ROWS_PER_CORE - lo)
                        if ci < N_WIN:  # chunk 0: overwrite
                            nc.vector.tensor_copy(out=acc[:, lo:lo + wlen],
                                                  in_=cur_psum[:, :wlen])
                        else:
                            nc.vector.tensor_tensor(out=acc[:, lo:lo + wlen],
                                                    in0=cur_psum[:, :wlen],
                                                    in1=acc[:, lo:lo + wlen],
                                                    op=addop)
                        if ci >= (N_CHUNK - 1) * N_WIN:  # last chunk: epilogue
                            epilogue(w)

    nc.finalize()
    return nc


def kernel(embeddings, adj_vals, W1, b1, W2, b2, adj_rows, adj_cols):
    from concourse.bass_utils import run_bass_kernel_spmd

    embeddings = np.asarray(embeddings, dtype=np.float32)
    W1 = np.asarray(W1, dtype=np.float32)
    W2 = np.asarray(W2, dtype=np.float32)
    b1 = np.asarray(b1, dtype=np.float32)
    b2 = np.asarray(b2, dtype=np.float32)

    meta, idx_wrapped, rl_all, val_all = _prep(embeddings, adj_vals,
                                               adj_rows, adj_cols)

    # chunk tables (padded to CHUNK rows except last)
    chunk_rows = []
    chunks = []
    for ch in range(N_CHUNK):
        lo = ch * CHUNK
        hi = min(lo + CHUNK, N_NODES)
        chunks.append(np.ascontiguousarray(embeddings[lo:hi]))
        chunk_rows.append(hi - lo)

    key = (meta["tt"], tuple(meta["ops"]))
    if key not in _BUILD_CACHE:
        _BUILD_CACHE[key] = _build(meta, chunk_rows)
    nc = _BUILD_CACHE[key]

    bias = (b1 + 2.0 * b2).astype(np.float32).reshape(D, 1)
    w1t = np.ascontiguousarray(W1.T)
    w2t = np.ascontiguousarray(W2.T)

    in_maps = []
    for c in range(N_CORES):
        im = {f"embc{ch}": chunks[ch] for ch in range(N_CHUNK)}
        im["idx"] = idx_wrapped[c]
        im["rl"] = rl_all[c]
        im["val"] = val_all[c]
        im["embt"] = np.ascontiguousarray(
            embeddings[c * ROWS_PER_CORE:(c + 1) * ROWS_PER_CORE].T)
        im["w1t"] = w1t
        im["w2t"] = w2t
        im["bias"] = bias
        im["b2v"] = b2.astype(np.float32).reshape(D, 1)
        in_maps.append(im)

    res = run_bass_kernel_spmd(nc, in_maps, core_ids=list(range(N_CORES)))

    out = np.empty((N_NODES, D), dtype=np.float32)
    for c in range(N_CORES):
        out[c * ROWS_PER_CORE:(c + 1) * ROWS_PER_CORE] = res.results[c]["outt"].T
    return out


# revision 8
# speedup vs baseline: 1.7781x; 1.7781x over previous
"""NGCF layer (gather + segment_sum + dense epilogue) on 8 Trainium2 cores.

Strategy:
- Shard destination nodes (rows) across 8 cores: core c owns rows
  [c*12500, (c+1)*12500).
- Per core, edges are grouped by (source col chunk of 25088 rows,
  dest window of 128 rows), padded per cell to a multiple of 128 and to
  a cross-core-uniform tile count (SPMD: one program, per-core data).
- Messages are gathered from HBM via the custom dma_gather instruction
  (int16 indices per 25088-row chunk table, 1024-index ops spread over
  4 SWDGE queues for parallel descriptor generation).
- Each 128-edge tile is scattered into its window via a PE matmul with a
  DVE-built one-hot matrix S[e, r] = val_e * (rl_e == r); windows
  accumulate in PSUM and flush into an SBUF accumulator holding
  neighbor^T [128 dims, 12500 rows].
- Dense epilogue per window, all in transposed layout:
  out^T = leaky_relu(W1^T.T @ X^T + W2^T.T @ N^T + W2^T.T @ ((W2^T.T @ N^T) * X^T) + bias)
- Host assembles the full [100000, 128] output.
"""

import sys

sys.path.insert(0, "/opt/trn_rl_repo")

import numpy as np

N_NODES = 100000
N_EDGES = 1600000
D = 128
N_CORES = 8
ROWS_PER_CORE = N_NODES // N_CORES          # 12500
WIN = 128                                    # dest rows per window
N_WIN = (ROWS_PER_CORE + WIN - 1) // WIN     # 98 (last window 84 rows)
CHUNK = 25088                                # source rows per gather table chunk
N_CHUNK = (N_NODES + CHUNK - 1) // CHUNK     # 4
TILE = 128                                   # edges per matmul tile
OP_TILES = 8                                 # tiles per dma_gather op (1024 idx)


def _prep(embeddings, adj_vals, adj_rows, adj_cols):
    """Host-side: shard + sort edges, build padded per-core streams."""
    rows = np.asarray(adj_rows).astype(np.int64)
    cols = np.asarray(adj_cols).astype(np.int64)
    vals = np.asarray(adj_vals).astype(np.float32)

    core = rows // ROWS_PER_CORE
    rows_l = rows - core * ROWS_PER_CORE
    win = rows_l // WIN
    chunk = cols // CHUNK
    cell = chunk * N_WIN + win                       # cell id within a core
    n_cells = N_CHUNK * N_WIN

    per_core = []
    cnt = np.zeros((N_CORES, n_cells), dtype=np.int64)
    for c in range(N_CORES):
        m = core == c
        order = np.lexsort((rows_l[m], cell[m]))
        e = {
            "cell": cell[m][order],
            "rl": (rows_l[m] - win[m] * WIN)[order].astype(np.float32),
            "idx": (cols[m] - chunk[m] * CHUNK)[order].astype(np.int16),
            "val": vals[m][order],
        }
        cnt[c] = np.bincount(e["cell"], minlength=n_cells)
        per_core.append(e)

    # uniform cell capacities (tiles) across cores
    tiles_per_cell = np.maximum(1, -(-cnt.max(axis=0) // TILE))   # [n_cells]
    cap = tiles_per_cell * TILE
    cell_start = np.concatenate([[0], np.cumsum(cap)])            # [n_cells+1]
    total = int(cell_start[-1])                                   # padded edges/core
    tt = total // TILE                                            # total tiles/core

    # per-chunk tile counts and gather op sizes (same for all cores)
    chunk_tiles = [int(tiles_per_cell[ch * N_WIN:(ch + 1) * N_WIN].sum())
                   for ch in range(N_CHUNK)]
    ops = []   # (chunk, tile_offset_global, n_tiles)
    toff = 0
    for ch in range(N_CHUNK):
        left = chunk_tiles[ch]
        while left > 0:
            n = min(OP_TILES, left)
            ops.append((ch, toff, n))
            toff += n
            left -= n
    assert toff == tt

    # per-tile cell id / flags (same for all cores)
    tile_cell = np.repeat(np.arange(n_cells), tiles_per_cell)
    first = np.zeros(tt, dtype=bool)
    last = np.zeros(tt, dtype=bool)
    pos = 0
    for ci in range(n_cells):
        k = tiles_per_cell[ci]
        first[pos] = True
        last[pos + k - 1] = True
        pos += k

    # per-core padded streams
    idx_streams, rl_all, val_all = [], [], []
    for c in range(N_CORES):
        e = per_core[c]
        n = len(e["cell"])
        # slot of each real edge: cell_start[cell] + rank within cell
        cell_first = np.searchsorted(e["cell"], np.arange(n_cells), side="left")
        rank = np.arange(n) - cell_first[e["cell"]]
        slot = cell_start[e["cell"]] + rank
        idx_s = np.zeros(total, dtype=np.int16)
        rl_s = np.zeros(total, dtype=np.float32)
        val_s = np.zeros(total, dtype=np.float32)
        idx_s[slot] = e["idx"]
        rl_s[slot] = e["rl"]
        val_s[slot] = e["val"]
        idx_streams.append(idx_s)
        rl_all.append(rl_s.reshape(tt, TILE).T.copy())      # [128, tt]
        val_all.append(val_s.reshape(tt, TILE).T.copy())    # [128, tt]

    # idx arrays in the dma_gather wrapped layout, one column-block per op
    idx_wrapped = []
    for c in range(N_CORES):
        parts = []
        for (ch, toff2, ntile) in ops:
            chunk_idx = idx_streams[c][toff2 * TILE:(toff2 + ntile) * TILE]
            parts.append(chunk_idx.reshape(-1, 16).T)       # [16, ntile*8]
        w16 = np.concatenate(parts, axis=1)                 # [16, tt*8]
        idx_wrapped.append(np.tile(w16, (8, 1)))            # [128, tt*8]

    meta = {
        "tt": tt,
        "ops": ops,
        "tile_cell": tile_cell,
        "first": first,
        "last": last,
        "tiles_per_cell": tiles_per_cell,
    }
    return meta, idx_wrapped, rl_all, val_all


_BUILD_CACHE = {}


def _build(meta, chunk_rows):
    import concourse.bacc as bacc
    import concourse.bass as bass
    import concourse.mybir as mybir
    import concourse.tile as tile
    from concourse.library_config import mlp

    tt = meta["tt"]
    ops = meta["ops"]
    tile_cell = meta["tile_cell"]
    first = meta["first"]
    last = meta["last"]

    nc = bacc.Bacc(None, target_bir_lowering=False, num_swdge_queues=4)
    f32 = mybir.dt.float32
    embc = [nc.dram_tensor(f"embc{ch}", [chunk_rows[ch], D], f32, kind="ExternalInput")
            for ch in range(N_CHUNK)]
    idx_t = nc.dram_tensor("idx", [128, tt * 8], mybir.dt.int16, kind="ExternalInput")
    rl_t = nc.dram_tensor("rl", [128, tt], f32, kind="ExternalInput")
    val_t = nc.dram_tensor("val", [128, tt], f32, kind="ExternalInput")
    embt_t = nc.dram_tensor("embt", [128, ROWS_PER_CORE], f32, kind="ExternalInput")
    w1t_t = nc.dram_tensor("w1t", [D, D], f32, kind="ExternalInput")
    w2t_t = nc.dram_tensor("w2t", [D, D], f32, kind="ExternalInput")
    bias_t = nc.dram_tensor("bias", [D, 1], f32, kind="ExternalInput")
    b2_t = nc.dram_tensor("b2v", [D, 1], f32, kind="ExternalInput")
    outt_t = nc.dram_tensor("outt", [128, ROWS_PER_CORE], f32, kind="ExternalOutput")

    eq = mybir.AluOpType.is_equal
    mult = mybir.AluOpType.mult
    addop = mybir.AluOpType.add
    maxop = mybir.AluOpType.max

    with tile.TileContext(nc) as tc:
        with (
            tc.tile_pool(name="const", bufs=1) as constp,
            tc.tile_pool(name="gath", bufs=4) as gathp,
            tc.tile_pool(name="stp", bufs=3) as stp,
            tc.tile_pool(name="pcell", bufs=4, space="PSUM") as pcell,
            tc.tile_pool(name="pepi", bufs=2, space="PSUM") as pepi,
            tc.tile_pool(name="episb", bufs=3) as episb,
        ):
            nc.gpsimd.load_library(mlp)

            idx_sb = constp.tile([128, tt * 8], mybir.dt.int16)
            nc.sync.dma_start(out=idx_sb[:], in_=idx_t[:])
            rl_sb = constp.tile([128, tt], f32)
            nc.sync.dma_start(out=rl_sb[:], in_=rl_t[:])
            val_sb = constp.tile([128, tt], f32)
            nc.sync.dma_start(out=val_sb[:], in_=val_t[:])
            w1t_sb = constp.tile([D, D], f32)
            nc.sync.dma_start(out=w1t_sb[:], in_=w1t_t[:])
            w2t_sb = constp.tile([D, D], f32)
            nc.sync.dma_start(out=w2t_sb[:], in_=w2t_t[:])
            bias_sb = constp.tile([D, 1], f32)
            nc.sync.dma_start(out=bias_sb[:], in_=bias_t[:])
            b2_sb = constp.tile([D, 1], f32)
            nc.sync.dma_start(out=b2_sb[:], in_=b2_t[:])

            acc = constp.tile([128, ROWS_PER_CORE], f32)

            # iota 0..127 repeated OP_TILES times along free: [128, 8, 128]
            iota_i = constp.tile([128, OP_TILES * TILE], mybir.dt.int32)
            nc.gpsimd.iota(iota_i[:], pattern=[[0, OP_TILES], [1, TILE]],
                           base=0, channel_multiplier=0)
            iota_f = constp.tile([128, OP_TILES * TILE], f32)
            nc.vector.tensor_copy(out=iota_f[:], in_=iota_i[:])

            def epilogue(w):
                lo = w * WIN
                wlen = min(WIN, ROWS_PER_CORE - lo)
                xs_t = episb.tile([128, WIN], f32, tag="xs")
                nc.sync.dma_start(out=xs_t[:, :wlen], in_=embt_t[:, lo:lo + wlen])
                xs = xs_t[:, :wlen]
                pa = pepi.tile([128, WIN], f32, tag="pa")
                nc.tensor.matmul(out=pa[:, :wlen], lhsT=w2t_sb[:],
                                 rhs=acc[:, lo:lo + wlen], start=True, stop=True)
                it = episb.tile([128, WIN], f32, tag="it")
                nc.vector.scalar_tensor_tensor(out=it[:, :wlen], in0=pa[:, :wlen],
                                               scalar=b2_sb[:], in1=xs,
                                               op0=addop, op1=mult)
                pb = pepi.tile([128, WIN], f32, tag="pb")
                nc.tensor.matmul(out=pb[:, :wlen], lhsT=w1t_sb[:], rhs=xs,
                                 start=True, stop=False)
                nc.tensor.matmul(out=pb[:, :wlen], lhsT=w2t_sb[:],
                                 rhs=acc[:, lo:lo + wlen], start=False, stop=False)
                nc.tensor.matmul(out=pb[:, :wlen], lhsT=w2t_sb[:],
                                 rhs=it[:, :wlen], start=False, stop=True)
                s2 = episb.tile([128, WIN], f32, tag="s2")
                nc.vector.tensor_scalar_add(s2[:, :wlen], pb[:, :wlen], bias_sb[:])
                t1 = episb.tile([128, WIN], f32, tag="t1")
                nc.scalar.mul(t1[:, :wlen], s2[:, :wlen], 0.2)
                o = episb.tile([128, WIN], f32, tag="o")
                nc.vector.tensor_tensor(out=o[:, :wlen], in0=s2[:, :wlen],
                                        in1=t1[:, :wlen], op=maxop)
                nc.sync.dma_start(out=outt_t[:, lo:lo + wlen], in_=o[:, :wlen])

            cur_psum = None
            qn = 0
            for (ch, toff, ntile) in ops:
                n_idx = ntile * TILE
                dst = gathp.tile([128, OP_TILES, D], f32, tag="dst")
                nc.gpsimd.dma_gather(
                    dst[:, :ntile, :],
                    embc[ch][:],
                    idx_sb[:, toff * 8:toff * 8 + n_idx // 16],
                    n_idx, n_idx, D,
                    single_packet=True,
                    queue_num=qn % 4,
                )
                qn += 1
                # one wide S build for the whole op
                st_big = stp.tile([128, OP_TILES * TILE], f32, tag="st")
                rl_b = rl_sb[:, toff:toff + ntile].to_broadcast(
                    (128, ntile, TILE))
                val_b = val_sb[:, toff:toff + ntile].to_broadcast(
                    (128, ntile, TILE))
                eqv = stp.tile([128, OP_TILES * TILE], f32, tag="eqv")
                nc.vector.tensor_tensor(
                    out=eqv[:, :n_idx].rearrange("p (a b) -> p a b", b=TILE),
                    in0=iota_f[:, :n_idx].rearrange("p (a b) -> p a b", b=TILE),
                    in1=rl_b, op=eq)
                nc.vector.tensor_tensor(
                    out=st_big[:, :n_idx].rearrange("p (a b) -> p a b", b=TILE),
                    in0=eqv[:, :n_idx].rearrange("p (a b) -> p a b", b=TILE),
                    in1=val_b, op=mult)

                for j in range(ntile):
                    t = toff + j
                    ci = int(tile_cell[t])
                    w = ci % N_WIN
                    if first[t]:
                        cur_psum = pcell.tile([128, WIN], f32, tag="pc")
                    nc.tensor.matmul(
                        out=cur_psum[:],
                        lhsT=dst[:, j, :],
                        rhs=st_big[:, j * TILE:(j + 1) * TILE],
                        start=bool(first[t]), stop=bool(last[t]),
                    )
                    if last[t]:
                        lo = w * WIN
                        wlen = min(WIN, ROWS_PER_CORE - lo)
                        if ci < N_WIN:  # chunk 0: overwrite
                            nc.vector.tensor_copy(out=acc[:, lo:lo + wlen],
                                                  in_=cur_psum[:, :wlen])
                        else:
                            nc.vector.tensor_tensor(out=acc[:, lo:lo + wlen],
                                                    in0=cur_psum[:, :wlen],
                                                    in1=acc[:, lo:lo + wlen],
                                                    op=addop)
                        if ci >= (N_CHUNK - 1) * N_WIN:  # last chunk: epilogue
                            epilogue(w)

    nc.finalize()
    return nc


def kernel(embeddings, adj_vals, W1, b1, W2, b2, adj_rows, adj_cols):
    from concourse.bass_utils import run_bass_kernel_spmd

    embeddings = np.asarray(embeddings, dtype=np.float32)
    W1 = np.asarray(W1, dtype=np.float32)
    W2 = np.asarray(W2, dtype=np.float32)
    b1 = np.asarray(b1, dtype=np.float32)
    b2 = np.asarray(b2, dtype=np.float32)

    meta, idx_wrapped, rl_all, val_all = _prep(embeddings, adj_vals,
                                               adj_rows, adj_cols)

    # chunk tables (padded to CHUNK rows except last)
    chunk_rows = []
    chunks = []
    for ch in range(N_CHUNK):
        lo = ch * CHUNK
        hi = min(lo + CHUNK, N_NODES)
        chunks.append(np.ascontiguousarray(embeddings[lo:hi]))
        chunk_rows.append(hi - lo)

    key = (meta["tt"], tuple(meta["ops"]))
    if key not in _BUILD_CACHE:
        _BUILD_CACHE[key] = _build(meta, chunk_rows)
    nc = _BUILD_CACHE[key]

    bias = (b1 + 2.0 * b2).astype(np.float32).reshape(D, 1)
    w1t = np.ascontiguousarray(W1.T)
    w2t = np.ascontiguousarray(W2.T)

    in_maps = []
    for c in range(N_CORES):
        im = {f"embc{ch}": chunks[ch] for ch in range(N_CHUNK)}
        im["idx"] = idx_wrapped[c]
        im["rl"] = rl_all[c]
        im["val"] = val_all[c]
        im["embt"] = np.ascontiguousarray(
            embeddings[c * ROWS_PER_CORE:(c + 1) * ROWS_PER_CORE].T)
        im["w1t"] = w1t
        im["w2t"] = w2t
        im["bias"] = bias
        im["b2v"] = b2.astype(np.float32).reshape(D, 1)
        in_maps.append(im)

    try:
        res = run_bass_kernel_spmd(nc, in_maps, core_ids=list(range(N_CORES)))
    except Exception:
        # a previously wedged device usually recovers on the next attempt
        res = run_bass_kernel_spmd(nc, in_maps, core_ids=list(range(N_CORES)))

    out = np.empty((N_NODES, D), dtype=np.float32)
    for c in range(N_CORES):
        out[c * ROWS_PER_CORE:(c + 1) * ROWS_PER_CORE] = res.results[c]["outt"].T
    return out


# revision 9
# speedup vs baseline: 1.7827x; 1.0026x over previous
"""NGCF layer (gather + segment_sum + dense epilogue) on 8 Trainium2 cores.

Strategy:
- Shard destination nodes (rows) across 8 cores: core c owns rows
  [c*12500, (c+1)*12500).
- Per core, edges are grouped by (source col chunk of 25088 rows,
  dest window of 128 rows), padded per cell to a multiple of 128 and to
  a cross-core-uniform tile count (SPMD: one program, per-core data).
- Messages are gathered from HBM via the custom dma_gather instruction
  (int16 indices per 25088-row chunk table, 1024-index ops spread over
  4 SWDGE queues for parallel descriptor generation).
- Each 128-edge tile is scattered into its window via a PE matmul with a
  DVE-built one-hot matrix S[e, r] = val_e * (rl_e == r); windows
  accumulate in PSUM and flush into an SBUF accumulator holding
  neighbor^T [128 dims, 12500 rows].
- Dense epilogue per window, all in transposed layout:
  out^T = leaky_relu(W1^T.T @ X^T + W2^T.T @ N^T + W2^T.T @ ((W2^T.T @ N^T) * X^T) + bias)
- Host assembles the full [100000, 128] output.
"""

import sys

sys.path.insert(0, "/opt/trn_rl_repo")

import numpy as np

N_NODES = 100000
N_EDGES = 1600000
D = 128
N_CORES = 8
ROWS_PER_CORE = N_NODES // N_CORES          # 12500
WIN = 128                                    # dest rows per window
N_WIN = (ROWS_PER_CORE + WIN - 1) // WIN     # 98 (last window 84 rows)
CHUNK = 25088                                # source rows per gather table chunk
N_CHUNK = (N_NODES + CHUNK - 1) // CHUNK     # 4
TILE = 128                                   # edges per matmul tile
OP_TILES = 8                                 # tiles per dma_gather op (1024 idx)


def _prep(embeddings, adj_vals, adj_rows, adj_cols):
    """Host-side: shard + sort edges, build padded per-core streams."""
    rows = np.asarray(adj_rows).astype(np.int64)
    cols = np.asarray(adj_cols).astype(np.int64)
    vals = np.asarray(adj_vals).astype(np.float32)

    core = rows // ROWS_PER_CORE
    rows_l = rows - core * ROWS_PER_CORE
    win = rows_l // WIN
    chunk = cols // CHUNK
    cell = chunk * N_WIN + win                       # cell id within a core
    n_cells = N_CHUNK * N_WIN

    per_core = []
    cnt = np.zeros((N_CORES, n_cells), dtype=np.int64)
    for c in range(N_CORES):
        m = core == c
        order = np.lexsort((rows_l[m], cell[m]))
        e = {
            "cell": cell[m][order],
            "rl": (rows_l[m] - win[m] * WIN)[order].astype(np.float32),
            "idx": (cols[m] - chunk[m] * CHUNK)[order].astype(np.int16),
            "val": vals[m][order],
        }
        cnt[c] = np.bincount(e["cell"], minlength=n_cells)
        per_core.append(e)

    # uniform cell capacities (tiles) across cores
    tiles_per_cell = np.maximum(1, -(-cnt.max(axis=0) // TILE))   # [n_cells]
    cap = tiles_per_cell * TILE
    cell_start = np.concatenate([[0], np.cumsum(cap)])            # [n_cells+1]
    total = int(cell_start[-1])                                   # padded edges/core
    tt = total // TILE                                            # total tiles/core

    # per-chunk tile counts and gather op sizes (same for all cores)
    chunk_tiles = [int(tiles_per_cell[ch * N_WIN:(ch + 1) * N_WIN].sum())
                   for ch in range(N_CHUNK)]
    ops = []   # (chunk, tile_offset_global, n_tiles)
    toff = 0
    for ch in range(N_CHUNK):
        left = chunk_tiles[ch]
        while left > 0:
            n = min(OP_TILES, left)
            ops.append((ch, toff, n))
            toff += n
            left -= n
    assert toff == tt

    # per-tile cell id / flags (same for all cores)
    tile_cell = np.repeat(np.arange(n_cells), tiles_per_cell)
    first = np.zeros(tt, dtype=bool)
    last = np.zeros(tt, dtype=bool)
    pos = 0
    for ci in range(n_cells):
        k = tiles_per_cell[ci]
        first[pos] = True
        last[pos + k - 1] = True
        pos += k

    # per-core padded streams
    idx_streams, rl_all, val_all = [], [], []
    for c in range(N_CORES):
        e = per_core[c]
        n = len(e["cell"])
        # slot of each real edge: cell_start[cell] + rank within cell
        cell_first = np.searchsorted(e["cell"], np.arange(n_cells), side="left")
        rank = np.arange(n) - cell_first[e["cell"]]
        slot = cell_start[e["cell"]] + rank
        idx_s = np.zeros(total, dtype=np.int16)
        rl_s = np.zeros(total, dtype=np.float32)
        val_s = np.zeros(total, dtype=np.float32)
        idx_s[slot] = e["idx"]
        rl_s[slot] = e["rl"]
        val_s[slot] = e["val"]
        idx_streams.append(idx_s)
        rl_all.append(rl_s.reshape(tt, TILE).T.copy())      # [128, tt]
        val_all.append(val_s.reshape(tt, TILE).T.copy())    # [128, tt]

    # idx arrays in the dma_gather wrapped layout, one column-block per op
    idx_wrapped = []
    for c in range(N_CORES):
        parts = []
        for (ch, toff2, ntile) in ops:
            chunk_idx = idx_streams[c][toff2 * TILE:(toff2 + ntile) * TILE]
            parts.append(chunk_idx.reshape(-1, 16).T)       # [16, ntile*8]
        w16 = np.concatenate(parts, axis=1)                 # [16, tt*8]
        idx_wrapped.append(np.tile(w16, (8, 1)))            # [128, tt*8]

    meta = {
        "tt": tt,
        "ops": ops,
        "tile_cell": tile_cell,
        "first": first,
        "last": last,
        "tiles_per_cell": tiles_per_cell,
    }
    return meta, idx_wrapped, rl_all, val_all


_BUILD_CACHE = {}


def _build(meta, chunk_rows):
    import concourse.bacc as bacc
    import concourse.bass as bass
    import concourse.mybir as mybir
    import concourse.tile as tile
    from concourse.library_config import mlp

    tt = meta["tt"]
    ops = meta["ops"]
    tile_cell = meta["tile_cell"]
    first = meta["first"]
    last = meta["last"]

    nc = bacc.Bacc(None, target_bir_lowering=False, num_swdge_queues=4)
    f32 = mybir.dt.float32
    embc = [nc.dram_tensor(f"embc{ch}", [chunk_rows[ch], D], f32, kind="ExternalInput")
            for ch in range(N_CHUNK)]
    idx_t = nc.dram_tensor("idx", [128, tt * 8], mybir.dt.int16, kind="ExternalInput")
    rl_t = nc.dram_tensor("rl", [128, tt], f32, kind="ExternalInput")
    val_t = nc.dram_tensor("val", [128, tt], f32, kind="ExternalInput")
    embt_t = nc.dram_tensor("embt", [128, ROWS_PER_CORE], f32, kind="ExternalInput")
    w1t_t = nc.dram_tensor("w1t", [D, D], f32, kind="ExternalInput")
    w2t_t = nc.dram_tensor("w2t", [D, D], f32, kind="ExternalInput")
    bias_t = nc.dram_tensor("bias", [D, 1], f32, kind="ExternalInput")
    b2_t = nc.dram_tensor("b2v", [D, 1], f32, kind="ExternalInput")
    outt_t = nc.dram_tensor("outt", [128, ROWS_PER_CORE], f32, kind="ExternalOutput")

    eq = mybir.AluOpType.is_equal
    mult = mybir.AluOpType.mult
    addop = mybir.AluOpType.add
    maxop = mybir.AluOpType.max

    with tile.TileContext(nc) as tc:
        with (
            tc.tile_pool(name="const", bufs=1) as constp,
            tc.tile_pool(name="gath", bufs=6) as gathp,
            tc.tile_pool(name="stp", bufs=3) as stp,
            tc.tile_pool(name="pcell", bufs=4, space="PSUM") as pcell,
            tc.tile_pool(name="pepi", bufs=2, space="PSUM") as pepi,
            tc.tile_pool(name="episb", bufs=3) as episb,
        ):
            nc.gpsimd.load_library(mlp)

            idx_sb = constp.tile([128, tt * 8], mybir.dt.int16)
            nc.sync.dma_start(out=idx_sb[:], in_=idx_t[:])
            rl_sb = constp.tile([128, tt], f32)
            nc.sync.dma_start(out=rl_sb[:], in_=rl_t[:])
            val_sb = constp.tile([128, tt], f32)
            nc.sync.dma_start(out=val_sb[:], in_=val_t[:])
            w1t_sb = constp.tile([D, D], f32)
            nc.sync.dma_start(out=w1t_sb[:], in_=w1t_t[:])
            w2t_sb = constp.tile([D, D], f32)
            nc.sync.dma_start(out=w2t_sb[:], in_=w2t_t[:])
            bias_sb = constp.tile([D, 1], f32)
            nc.sync.dma_start(out=bias_sb[:], in_=bias_t[:])
            b2_sb = constp.tile([D, 1], f32)
            nc.sync.dma_start(out=b2_sb[:], in_=b2_t[:])

            acc = constp.tile([128, ROWS_PER_CORE], f32)

            # iota 0..127 repeated OP_TILES times along free: [128, 8, 128]
            iota_i = constp.tile([128, OP_TILES * TILE], mybir.dt.int32)
            nc.gpsimd.iota(iota_i[:], pattern=[[0, OP_TILES], [1, TILE]],
                           base=0, channel_multiplier=0)
            iota_f = constp.tile([128, OP_TILES * TILE], f32)
            nc.vector.tensor_copy(out=iota_f[:], in_=iota_i[:])

            def epilogue(w):
                lo = w * WIN
                wlen = min(WIN, ROWS_PER_CORE - lo)
                xs_t = episb.tile([128, WIN], f32, tag="xs")
                nc.sync.dma_start(out=xs_t[:, :wlen], in_=embt_t[:, lo:lo + wlen])
                xs = xs_t[:, :wlen]
                pa = pepi.tile([128, WIN], f32, tag="pa")
                nc.tensor.matmul(out=pa[:, :wlen], lhsT=w2t_sb[:],
                                 rhs=acc[:, lo:lo + wlen], start=True, stop=True)
                it = episb.tile([128, WIN], f32, tag="it")
                nc.vector.scalar_tensor_tensor(out=it[:, :wlen], in0=pa[:, :wlen],
                                               scalar=b2_sb[:], in1=xs,
                                               op0=addop, op1=mult)
                pb = pepi.tile([128, WIN], f32, tag="pb")
                nc.tensor.matmul(out=pb[:, :wlen], lhsT=w1t_sb[:], rhs=xs,
                                 start=True, stop=False)
                nc.tensor.matmul(out=pb[:, :wlen], lhsT=w2t_sb[:],
                                 rhs=acc[:, lo:lo + wlen], start=False, stop=False)
                nc.tensor.matmul(out=pb[:, :wlen], lhsT=w2t_sb[:],
                                 rhs=it[:, :wlen], start=False, stop=True)
                s2 = episb.tile([128, WIN], f32, tag="s2")
                nc.vector.tensor_scalar_add(s2[:, :wlen], pb[:, :wlen], bias_sb[:])
                t1 = episb.tile([128, WIN], f32, tag="t1")
                nc.scalar.mul(t1[:, :wlen], s2[:, :wlen], 0.2)
                o = episb.tile([128, WIN], f32, tag="o")
                nc.vector.tensor_tensor(out=o[:, :wlen], in0=s2[:, :wlen],
                                        in1=t1[:, :wlen], op=maxop)
                nc.sync.dma_start(out=outt_t[:, lo:lo + wlen], in_=o[:, :wlen])

            cur_psum = None
            qn = 0
            for (ch, toff, ntile) in ops:
                n_idx = ntile * TILE
                dst = gathp.tile([128, OP_TILES, D], f32, tag="dst")
                nc.gpsimd.dma_gather(
                    dst[:, :ntile, :],
                    embc[ch][:],
                    idx_sb[:, toff * 8:toff * 8 + n_idx // 16],
                    n_idx, n_idx, D,
                    single_packet=True,
                    queue_num=qn % 4,
                )
                qn += 1
                # one wide S build for the whole op
                st_big = stp.tile([128, OP_TILES * TILE], f32, tag="st")
                rl_b = rl_sb[:, toff:toff + ntile].to_broadcast(
                    (128, ntile, TILE))
                val_b = val_sb[:, toff:toff + ntile].to_broadcast(
                    (128, ntile, TILE))
                eqv = stp.tile([128, OP_TILES * TILE], f32, tag="eqv")
                nc.vector.tensor_tensor(
                    out=eqv[:, :n_idx].rearrange("p (a b) -> p a b", b=TILE),
                    in0=iota_f[:, :n_idx].rearrange("p (a b) -> p a b", b=TILE),
                    in1=rl_b, op=eq)
                nc.vector.tensor_tensor(
                    out=st_big[:, :n_idx].rearrange("p (a b) -> p a b", b=TILE),
                    in0=eqv[:, :n_idx].rearrange("p (a b) -> p a b", b=TILE),
                    in1=val_b, op=mult)

                for j in range(ntile):
                    t = toff + j
                    ci = int(tile_cell[t])
                    w = ci % N_WIN
                    if first[t]:
                        cur_psum = pcell.tile([128, WIN], f32, tag="pc")
                    nc.tensor.matmul(
                        out=cur_psum[:],
                        lhsT=dst[:, j, :],
                        rhs=st_big[:, j * TILE:(j + 1) * TILE],
                        start=bool(first[t]), stop=bool(last[t]),
                    )
                    if last[t]:
                        lo = w * WIN
                        wlen = min(WIN, ROWS_PER_CORE - lo)
                        if ci < N_WIN:  # chunk 0: overwrite
                            nc.vector.tensor_copy(out=acc[:, lo:lo + wlen],
                                                  in_=cur_psum[:, :wlen])
                        else:
                            nc.vector.tensor_tensor(out=acc[:, lo:lo + wlen],
                                                    in0=cur_psum[:, :wlen],
                                                    in1=acc[:, lo:lo + wlen],
                                                    op=addop)
                        if ci >= (N_CHUNK - 1) * N_WIN:  # last chunk: epilogue
                            epilogue(w)

    nc.finalize()
    return nc


def kernel(embeddings, adj_vals, W1, b1, W2, b2, adj_rows, adj_cols):
    from concourse.bass_utils import run_bass_kernel_spmd

    embeddings = np.asarray(embeddings, dtype=np.float32)
    W1 = np.asarray(W1, dtype=np.float32)
    W2 = np.asarray(W2, dtype=np.float32)
    b1 = np.asarray(b1, dtype=np.float32)
    b2 = np.asarray(b2, dtype=np.float32)

    meta, idx_wrapped, rl_all, val_all = _prep(embeddings, adj_vals,
                                               adj_rows, adj_cols)

    # chunk tables (padded to CHUNK rows except last)
    chunk_rows = []
    chunks = []
    for ch in range(N_CHUNK):
        lo = ch * CHUNK
        hi = min(lo + CHUNK, N_NODES)
        chunks.append(np.ascontiguousarray(embeddings[lo:hi]))
        chunk_rows.append(hi - lo)

    key = (meta["tt"], tuple(meta["ops"]))
    if key not in _BUILD_CACHE:
        _BUILD_CACHE[key] = _build(meta, chunk_rows)
    nc = _BUILD_CACHE[key]

    bias = (b1 + 2.0 * b2).astype(np.float32).reshape(D, 1)
    w1t = np.ascontiguousarray(W1.T)
    w2t = np.ascontiguousarray(W2.T)

    in_maps = []
    for c in range(N_CORES):
        im = {f"embc{ch}": chunks[ch] for ch in range(N_CHUNK)}
        im["idx"] = idx_wrapped[c]
        im["rl"] = rl_all[c]
        im["val"] = val_all[c]
        im["embt"] = np.ascontiguousarray(
            embeddings[c * ROWS_PER_CORE:(c + 1) * ROWS_PER_CORE].T)
        im["w1t"] = w1t
        im["w2t"] = w2t
        im["bias"] = bias
        im["b2v"] = b2.astype(np.float32).reshape(D, 1)
        in_maps.append(im)

    try:
        res = run_bass_kernel_spmd(nc, in_maps, core_ids=list(range(N_CORES)))
    except Exception:
        # a previously wedged device usually recovers on the next attempt
        res = run_bass_kernel_spmd(nc, in_maps, core_ids=list(range(N_CORES)))

    out = np.empty((N_NODES, D), dtype=np.float32)
    for c in range(N_CORES):
        out[c * ROWS_PER_CORE:(c + 1) * ROWS_PER_CORE] = res.results[c]["outt"].T
    return out
